# revision 20
# baseline (speedup 1.0000x reference)
"""Trainium2 Bass kernel for ActionPrototypeLayer (retrieval_knn).

Problem math (B=16384, A=64 app types, P=32 prototypes each, D=1024):
  sel    = prototypes[ids]                       [B,P,D]
  scores = (pooled . sel) / sqrt(D)              [B,P]
  attn   = softmax(scores)                       [B,P]
  action = LayerNorm(attn @ sel + pooled)        [B,D]
  sim    = (norm(pooled) . norm(protos)) / TEMP  [B,A,P]
  loss   = max(mean(-log(pos/total)), 0)   pos/total from exp(sim)

Everything derives from one dense matmul G = pooled @ protos_flat^T
([B,1024] x [1024, 2048]).  protos_flat^T is pre-scaled by 1/||proto_j||
so exp(sim) comes straight out of PSUM via one ScalarE activation with a
per-partition scale (1/(||pooled_b||*TEMP)) and a fused row-sum (total).

Sharding: data-parallel over batch, 2048 rows/core on 8 cores; prototypes
replicated.  The host SORTS the batch by app_type_id before sharding (a
pure permutation, undone on unshard), so each 128-row tile's rows select
prototypes from only 1-2 contiguous 128-column chunks ("resident window").
The attention softmax, mask+reduce ops, and the second (attention-weighted)
matmul run only on that window instead of all 2048 columns.

Engine-placement notes:
  - All ScalarE activations stay inside ONE table set
    (natural_log_exp_and_others: exp/ln/square/copy/identity).  rsqrt and
    sqrt are computed as exp(+-0.5*ln(x)) to avoid ~1.3us table reloads.
  - pooled/protos are fed as bf16 from the host so transposed operand
    layouts come straight from DMA-transpose (2-byte dtype requirement);
    no PE transposes or PSUM bounce copies for them.
  - Squares for norms use VectorE scalar_tensor_tensor self-multiplies
    with fused accumulation (bf16 4x mode).

Per-core loss partial is summed on host during unshard.
"""

import numpy as np

import concourse.bass as bass
import concourse.tile as tile
from concourse import mybir
from concourse.bass import MemorySpace
from concourse.bass_utils import run_bass_kernel_spmd
from concourse.masks import make_identity

B, A, P, D = 16384, 64, 32, 1024
NCORES = 8
BL = B // NCORES              # rows per core
AP_ = A * P                   # 2048 flattened prototypes
NDC = D // 128                # 8 contraction chunks
NJC = AP_ // 128              # 16 prototype-row chunks
TEMP = 0.07
SQD = float(np.sqrt(D))
LN_EPS = 1e-5

F32 = mybir.dt.float32
BF16 = mybir.dt.bfloat16
AF = mybir.ActivationFunctionType
OP = mybir.AluOpType
AX = mybir.AxisListType


def _bcast_row(ap, parts, n):
    """AP reading a [1, n] DRAM view broadcast across `parts` partitions."""
    return bass.AP(tensor=ap.tensor, offset=ap.offset, ap=[[0, parts], [1, n]])


def build_nc(bl=BL, jsets=None, trivial_gamma=False, trivial_beta=False,
             split_waits=True):
    """jsets: per-batch-tile (jc_lo, jc_hi) resident prototype-chunk range
    (inclusive).  None -> dense (0, NJC-1) for every tile."""
    nbt = bl // 128
    if jsets is None:
        jsets = [(0, NJC - 1)] * nbt
    assert len(jsets) == nbt
    wmax = max(hi - lo + 1 for lo, hi in jsets) * 128

    nc = bass.Bass()

    pooled = nc.declare_dram_parameter("pooled_bf", [bl, D], BF16, isOutput=False)
    pooledT = nc.declare_dram_parameter("pooledT_bf", [D, bl], BF16, isOutput=False)
    protos = nc.declare_dram_parameter("protos_bf", [AP_, D], BF16, isOutput=False)
    protosT = nc.declare_dram_parameter("protosT_bf", [D, AP_], BF16, isOutput=False)
    idsf = nc.declare_dram_parameter("idsf", [bl, 1], F32, isOutput=False)
    jblock = nc.declare_dram_parameter("jblock", [1, AP_], F32, isOutput=False)
    gamma = nc.declare_dram_parameter("gamma", [1, D], F32, isOutput=False)
    beta = nc.declare_dram_parameter("beta", [1, D], F32, isOutput=False)
    act_out = nc.declare_dram_parameter("act_out", [bl, D], F32, isOutput=True)
    loss_out = nc.declare_dram_parameter("loss_out", [1, 1], F32, isOutput=True)

    norms_dram = nc.dram_tensor("norms_scratch", [2, AP_], F32)

    with tile.TileContext(nc) as tc:
        with (
            tc.tile_pool(name="consts", bufs=1) as consts,
            tc.tile_pool(name="protop", bufs=1) as protop,
            tc.tile_pool(name="work", bufs=3) as work,
            tc.tile_pool(name="big", bufs=3 if wmax <= 512 else 1) as big,
            tc.tile_pool(name="small", bufs=3) as small,
            tc.tile_pool(name="psA", bufs=4, space=MemorySpace.PSUM) as psA,
            tc.tile_pool(name="psB", bufs=2, space=MemorySpace.PSUM) as psB,
            tc.tile_pool(name="psT", bufs=2, space=MemorySpace.PSUM) as psT,
        ):
            # ---------------- constants ----------------
            ident = consts.tile([128, 128], BF16, tag="ident")
            make_identity(nc, ident[:])
            ident_f = consts.tile([128, 128], F32, tag="ident_f")
            make_identity(nc, ident_f[:])
            jblock_b = consts.tile([128, AP_], BF16, tag="jblock_b")
            nc.gpsimd.dma_start(out=jblock_b[:], in_=_bcast_row(jblock[:, :], 128, AP_))
            if not trivial_gamma:
                gamma_b = consts.tile([128, D], F32, tag="gamma_b")
                nc.sync.dma_start(out=gamma_b[:], in_=_bcast_row(gamma[:, :], 128, D))
            if not trivial_beta:
                beta_b = consts.tile([128, D], F32, tag="beta_b")
                nc.sync.dma_start(out=beta_b[:], in_=_bcast_row(beta[:, :], 128, D))
            ones_col = consts.tile([128, 1], F32, tag="ones_col")
            nc.vector.memset(ones_col[:], 1.0)
            ones_row = consts.tile([1, 128], BF16, tag="ones_row")
            nc.vector.memset(ones_row[:], 1.0)
            loss_acc = consts.tile([128, 1], F32, tag="loss_acc")
            nc.vector.memset(loss_acc[:], 0.0)
            b_eps10 = consts.tile([128, 1], F32, tag="b_eps10")
            nc.vector.memset(b_eps10[:], 1e-10)
            b_lneps = consts.tile([128, 1], F32, tag="b_lneps")
            nc.vector.memset(b_lneps[:], LN_EPS)
            b_nlt = consts.tile([128, 1], F32, tag="b_nlt")
            nc.vector.memset(b_nlt[:], float(-np.log(TEMP)))

            # ---------------- prototype / pooled prep ----------------
            # proto_nat[jc]: [128 j, 1024 d] bf16 (action-matmul rhs)
            # protoT[dc]:    [128 d, 2048 j] bf16 via DMA-transpose, then
            #                columns pre-scaled by 1/||proto_j||
            # poolT_all[dc]: [128 d, bl b] bf16 via DMA-transpose
            proto_nat = [protop.tile([128, D], BF16, tag=f"pn{jc}", name=f"pn{jc}")
                         for jc in range(NJC)]
            protoT = [protop.tile([128, AP_], BF16, tag=f"pt{dc}", name=f"pt{dc}")
                      for dc in range(NDC)]
            poolT_all = [protop.tile([128, bl], BF16, tag=f"plT{dc}", name=f"plT{dc}")
                         for dc in range(NDC)]
            invn_all = consts.tile([128, NJC], F32, tag="invn_all")
            norm_all = consts.tile([128, NJC], F32, tag="norm_all")

            for dc in range(NDC):
                nc.scalar.dma_start(
                    out=protoT[dc][:], in_=protosT[dc * 128:(dc + 1) * 128, :])
                nc.scalar.dma_start(
                    out=poolT_all[dc][:], in_=pooledT[dc * 128:(dc + 1) * 128, :])

            for jc in range(NJC):
                nc.sync.dma_start(out=proto_nat[jc][:],
                                  in_=protos[jc * 128:(jc + 1) * 128, :])
                scr = work.tile([128, D], BF16, tag="scr")
                sqs = small.tile([128, 1], F32, tag="sqs")
                nc.scalar.activation(scr[:], proto_nat[jc][:], AF.Square,
                                     accum_out=sqs[:])
                lnz = small.tile([128, 1], F32, tag="lnz")
                nc.scalar.activation(lnz[:], sqs[:], AF.Ln)
                nc.scalar.activation(norm_all[:, jc:jc + 1], lnz[:], AF.Exp, scale=0.5)
                nc.scalar.activation(invn_all[:, jc:jc + 1], lnz[:], AF.Exp, scale=-0.5)

            # broadcast rows of 1/||p_j|| and ||p_j||: transpose [128,16] ->
            # [16,128] -> tiny DRAM bounce to flatten into a [1,2048] row ->
            # PE outer-products ones[128] x row-slab (partition broadcast)
            invnp_b = consts.tile([128, AP_], BF16, tag="invnp_b")
            nprot_b = consts.tile([128, AP_], BF16, tag="nprot_b")
            for row, srct, dstb in ((0, invn_all, invnp_b), (1, norm_all, nprot_b)):
                tpn = psT.tile([16, 128], F32, tag="tp", name=f"tpn{row}")
                nc.tensor.transpose(tpn[:], srct[:], ident_f[:])
                t16 = consts.tile([16, 128], F32, tag=f"t16_{row}", name=f"t16_{row}")
                nc.vector.tensor_copy(t16[:], tpn[:])
                nd = norms_dram[row:row + 1, :]
                dst = bass.AP(tensor=nd.tensor, offset=nd.offset, ap=[[128, 16], [1, 128]])
                nc.sync.dma_start(out=dst, in_=t16[:])
                rowt = consts.tile([1, AP_], BF16, tag=f"rowt{row}", name=f"rowt{row}")
                nc.gpsimd.dma_start(out=rowt[:], in_=nd)
                for s in range(4):
                    ob = psT.tile([128, 512], F32, tag="tp", name=f"ob{row}_{s}")
                    nc.tensor.matmul(ob[:], ones_row[:], rowt[0:1, s * 512:(s + 1) * 512],
                                     start=True, stop=True)
                    nc.scalar.copy(dstb[:, s * 512:(s + 1) * 512], ob[:])

            # pre-scale protoT columns by 1/||proto_j|| (in place)
            for dc in range(NDC):
                nc.vector.tensor_mul(protoT[dc][:], protoT[dc][:], invnp_b[:])

            # ---------------- per-batch-tile pipeline ----------------
            for bt in range(nbt):
                jlo, jhi = jsets[bt]
                nres = jhi - jlo + 1
                W = nres * 128
                jres = slice(jlo * 128, (jhi + 1) * 128)
                bsl = slice(bt * 128, (bt + 1) * 128)

                pool_nat = work.tile([128, D], BF16, tag="pool_nat")
                nc.sync.dma_start(out=pool_nat[:], in_=pooled[bsl, :])

                # ||pooled_b||^2 via DVE self-mult + accum;
                # escale = exp(-0.5*ln(sq) - ln(TEMP)) = 1/(||pooled||*TEMP)
                scr = work.tile([128, D], BF16, tag="scr")
                sqs = small.tile([128, 1], F32, tag="sqs")
                nc.vector.scalar_tensor_tensor(
                    scr[:], pool_nat[:], 1.0, pool_nat[:],
                    op0=OP.bypass, op1=OP.mult, accum_out=sqs[:])
                lnz = small.tile([128, 1], F32, tag="lnz")
                nc.scalar.activation(lnz[:], sqs[:], AF.Ln)
                escale = small.tile([128, 1], F32, tag="escale")
                nc.scalar.activation(escale[:], lnz[:], AF.Exp, scale=-0.5, bias=b_nlt[:])
                ids_c = small.tile([128, 1], F32, tag="ids_c")
                nc.sync.dma_start(out=ids_c[:], in_=idsf[bsl, :])

                # G matmul (columns pre-scaled by 1/||p_j||): 4 slabs of 512
                # E_c = exp(sim) straight from PSUM; per-slab row-sums
                E_c = big.tile([128, AP_], BF16, tag="E_c")
                tsl = small.tile([128, 4], F32, tag="tsl")
                gps = []
                for sl in range(4):
                    gp = psA.tile([128, 512], F32, tag="gp", name=f"gp{sl}")
                    gps.append(gp)
                    jsl = slice(sl * 512, (sl + 1) * 512)
                    for dc in range(NDC):
                        nc.tensor.matmul(
                            gp[:],
                            poolT_all[dc][:, bsl],
                            protoT[dc][:, jsl],
                            start=(dc == 0),
                            stop=(dc == NDC - 1),
                        )
                    nc.scalar.activation(E_c[:, jsl], gp[:], AF.Exp,
                                         scale=escale[:], accum_out=tsl[:, sl:sl + 1])
                # total = sum of slab partials
                t01 = small.tile([128, 1], F32, tag="t01")
                nc.vector.tensor_add(t01[:], tsl[:, 0:1], tsl[:, 1:2])
                t23 = small.tile([128, 1], F32, tag="t23")
                nc.vector.tensor_add(t23[:], tsl[:, 2:3], tsl[:, 3:4])
                total = small.tile([128, 1], F32, tag="total")
                nc.vector.tensor_add(total[:], t01[:], t23[:])

                # raw G on the resident window: G = gp * ||p_j||
                Graw = big.tile([128, wmax], F32, tag="Graw")
                for r in range(nres):
                    jc = jlo + r
                    sl = jc // 4
                    off = (jc % 4) * 128
                    nc.vector.tensor_mul(
                        Graw[:, r * 128:(r + 1) * 128],
                        gps[sl][:, off:off + 128],
                        nprot_b[:, jc * 128:(jc + 1) * 128],
                    )
                # attention numerators on the window
                E_a = big.tile([128, wmax], F32, tag="E_a")
                nc.scalar.activation(E_a[:, :W], Graw[:, :W], AF.Exp, scale=1.0 / SQD)

                # pos = sum over own block of E_c (mask fused)
                Ecm = big.tile([128, wmax], BF16, tag="Ecm")
                pos = small.tile([128, 1], F32, tag="pos")
                nc.vector.scalar_tensor_tensor(
                    Ecm[:, :W], jblock_b[:, jres], ids_c[:], E_c[:, jres],
                    op0=OP.is_equal, op1=OP.mult, accum_out=pos[:],
                )
                # attention: masked numerators + denominator
                E_am = big.tile([128, wmax], BF16, tag="E_am")
                den = small.tile([128, 1], F32, tag="den")
                nc.vector.scalar_tensor_tensor(
                    E_am[:, :W], jblock_b[:, jres], ids_c[:], E_a[:, :W],
                    op0=OP.is_equal, op1=OP.mult, accum_out=den[:],
                )

                # loss_acc += ln(total + 1e-10) - ln(pos)
                lt = small.tile([128, 1], F32, tag="lt")
                nc.scalar.activation(lt[:], total[:], AF.Ln, bias=b_eps10[:])
                lp = small.tile([128, 1], F32, tag="lp")
                nc.scalar.activation(lp[:], pos[:], AF.Ln)
                dlt = small.tile([128, 1], F32, tag="dlt")
                nc.vector.tensor_sub(dlt[:], lt[:], lp[:])
                nc.vector.tensor_add(loss_acc[:], loss_acc[:], dlt[:])

                # transpose E_am -> lhsT chunks
                E_amT = big.tile([128, wmax], BF16, tag="E_amT")
                for g in range((nres + 3) // 4):
                    kn = min(4, nres - g * 4)
                    tp = psT.tile([128, 4, 128], BF16, tag="tp")
                    for k in range(kn):
                        r = g * 4 + k
                        nc.tensor.transpose(tp[:, k], E_am[:, r * 128:(r + 1) * 128], ident[:])
                    nc.scalar.copy(
                        E_amT[:, g * 512:g * 512 + kn * 128].rearrange("p (k f) -> p k f", k=kn),
                        tp[:, 0:kn],
                    )

                # action matmul over resident chunks only;
                # act1 = U/den + pooled with fused row-sum (LayerNorm mean)
                recip_den = small.tile([128, 1], F32, tag="recip_den")
                nc.vector.reciprocal(recip_den[:], den[:])
                act1 = work.tile([128, D], F32, tag="act1")
                s1p = small.tile([128, 2], F32, tag="s1p")
                for dsl in range(2):
                    up = psB.tile([128, 512], F32, tag="up")
                    dslc = slice(dsl * 512, (dsl + 1) * 512)
                    for r in range(nres):
                        jc = jlo + r
                        nc.tensor.matmul(
                            up[:],
                            E_amT[:, r * 128:(r + 1) * 128],
                            proto_nat[jc][:, dslc],
                            start=(r == 0),
                            stop=(r == nres - 1),
                        )
                    nc.vector.scalar_tensor_tensor(
                        act1[:, dslc], up[:], recip_den[:], pool_nat[:, dslc],
                        op0=OP.mult, op1=OP.add, accum_out=s1p[:, dsl:dsl + 1],
                    )

                # LayerNorm(act1) * gamma + beta
                sum1 = small.tile([128, 1], F32, tag="sum1")
                nc.vector.tensor_add(sum1[:], s1p[:, 0:1], s1p[:, 1:2])
                negmu = small.tile([128, 1], F32, tag="negmu")
                nc.vector.tensor_scalar_mul(negmu[:], sum1[:], -1.0 / D)
                xc = work.tile([128, D], F32, tag="xc")
                nc.vector.tensor_scalar_add(xc[:], act1[:], negmu[:])
                scr2 = work.tile([128, D], BF16, tag="scr")
                vs = small.tile([128, 1], F32, tag="vs")
                nc.scalar.activation(scr2[:], xc[:], AF.Square, accum_out=vs[:])
                # rstd = exp(-0.5*ln(vs/D + eps))
                lnv = small.tile([128, 1], F32, tag="lnv")
                nc.scalar.activation(lnv[:], vs[:], AF.Ln, scale=1.0 / D, bias=b_lneps[:])
                rstd = small.tile([128, 1], F32, tag="rstd")
                nc.scalar.activation(rstd[:], lnv[:], AF.Exp, scale=-0.5)
                outt = work.tile([128, D], F32, tag="outt")
                if trivial_gamma:
                    nc.vector.tensor_scalar_mul(outt[:], xc[:], rstd[:])
                else:
                    nc.vector.scalar_tensor_tensor(
                        outt[:], xc[:], rstd[:], gamma_b[:],
                        op0=OP.mult, op1=OP.mult,
                    )
                if not trivial_beta:
                    nc.vector.tensor_add(outt[:], outt[:], beta_b[:])
                nc.scalar.dma_start(out=act_out[bsl, :], in_=outt[:])

            # ---------------- loss partial: partition-sum ----------------
            lps = psT.tile([1, 1], F32, tag="tp")
            nc.tensor.matmul(lps[:], loss_acc[:], ones_col[:], start=True, stop=True)
            lsb = small.tile([1, 1], F32, tag="lsb")
            nc.vector.tensor_copy(lsb[:], lps[:])
            nc.sync.dma_start(out=loss_out[:, :], in_=lsb[:])

    if split_waits:
        _split_multi_waits(nc)
    return nc


def _split_multi_waits(nc, max_cmds=2):
    """This walrus build allows at most ~2 sync commands (waits+updates) per
    instruction.  Tile emits up to 3+ waits on fan-in instructions; hoist the
    excess waits onto single-wait ENGINE_NOPs placed just before, on the same
    engine (same blocking semantics, engine streams run in program order)."""
    for fn in nc.m.functions:
        for blk in fn.blocks:
            new = []
            for inst in blk.instructions:
                si = getattr(inst, "sync_info", None)
                waits = list(si.on_wait) if si is not None and si.on_wait else []
                ups = list(si.on_update) if si is not None and si.on_update else []
                budget = min(1, max(0, max_cmds - len(ups)))
                if len(waits) > budget:
                    nkeep = budget
                    extra, kept = waits[:len(waits) - nkeep], waits[len(waits) - nkeep:]
                    for w in extra:
                        nop = mybir.InstEventSemaphore(
                            name=nc.get_next_instruction_name(),
                            engine=inst.engine,
                            ins=[],
                            outs=[],
                        )
                        nop.sync_info = mybir.SyncInfo(on_wait=[w], on_update=[])
                        new.append(nop)
                    inst.sync_info = mybir.SyncInfo(on_wait=kept, on_update=ups)
                new.append(inst)
            blk.instructions = new


_NC_CACHE = {}


def _get_nc(bl=BL, jsets=None, trivial_gamma=False, trivial_beta=False):
    key = (bl, tuple(jsets) if jsets is not None else None, trivial_gamma, trivial_beta)
    if key not in _NC_CACHE:
        _NC_CACHE[key] = build_nc(bl, jsets, trivial_gamma, trivial_beta)
    return _NC_CACHE[key]


def plan_shards(app_type_ids, ncores=NCORES, bl=BL):
    """Sort batch by id, then deal the 128-row sorted tiles round-robin to
    cores (core = g % ncores, slot = g // ncores).  Tile-slot s covers nearly
    the same id range on every core, so ONE SPMD graph (with the per-slot
    union of resident chunk ranges) serves all cores."""
    ids = np.asarray(app_type_ids).astype(np.int64).reshape(-1)
    base = np.argsort(ids, kind="stable")
    ids_sorted = ids[base]
    ngt = len(ids) // 128
    nslots = ngt // ncores
    order = []
    for c in range(ncores):
        for s in range(nslots):
            g = s * ncores + c
            order.append(base[128 * g:128 * (g + 1)])
    perm = np.concatenate(order)
    jsets = []
    for s in range(nslots):
        lo_id = int(ids_sorted[128 * (s * ncores)])
        hi_id = int(ids_sorted[128 * (s * ncores + ncores - 1) + 127])
        jsets.append((lo_id * P // 128, hi_id * P // 128))
    return perm, jsets


def make_in_maps(pooled_output, app_type_ids, prototypes, ln_gamma, ln_beta,
                 perm=None, ncores=NCORES, bl=BL):
    bf16 = mybir.dt.np(BF16)
    protoflat = np.asarray(prototypes, dtype=np.float32).reshape(AP_, D).astype(bf16)
    protoT = np.ascontiguousarray(protoflat.T)
    jblock = (np.arange(AP_, dtype=np.int64) // P).astype(np.float32).reshape(1, AP_)
    gamma = np.asarray(ln_gamma, dtype=np.float32).reshape(1, D)
    beta = np.asarray(ln_beta, dtype=np.float32).reshape(1, D)
    pooled_bf = np.asarray(pooled_output, dtype=np.float32).astype(bf16)
    idsf = np.asarray(app_type_ids).astype(np.float32).reshape(-1, 1)
    if perm is not None:
        pooled_bf = pooled_bf[perm]
        idsf = idsf[perm]
    in_maps = []
    for c in range(ncores):
        sl = slice(c * bl, (c + 1) * bl)
        in_maps.append({
            "pooled_bf": np.ascontiguousarray(pooled_bf[sl]),
            "pooledT_bf": np.ascontiguousarray(pooled_bf[sl].T),
            "protos_bf": protoflat,
            "protosT_bf": protoT,
            "idsf": np.ascontiguousarray(idsf[sl]),
            "jblock": jblock,
            "gamma": gamma,
            "beta": beta,
        })
    return in_maps


def _prep(pooled_output, app_type_ids, prototypes, ln_gamma, ln_beta):
    perm, jsets = plan_shards(app_type_ids)
    tg = bool(np.all(np.asarray(ln_gamma) == 1.0))
    tb = bool(np.all(np.asarray(ln_beta) == 0.0))
    nc = _get_nc(BL, jsets, tg, tb)
    in_maps = make_in_maps(pooled_output, app_type_ids, prototypes,
                           ln_gamma, ln_beta, perm=perm)
    return nc, in_maps, perm


def kernel(pooled_output, app_type_ids, prototypes, ln_gamma, ln_beta):
    nc, in_maps, perm = _prep(pooled_output, app_type_ids, prototypes,
                              ln_gamma, ln_beta)
    res = run_bass_kernel_spmd(nc, in_maps, core_ids=list(range(NCORES)))
    action_sorted = np.concatenate([r["act_out"] for r in res.results], axis=0)
    action = np.empty_like(action_sorted)
    action[perm] = action_sorted
    loss_sum = sum(float(r["loss_out"][0, 0]) for r in res.results)
    loss = np.float32(max(loss_sum / B, 0.0))
    return action.astype(np.float32), loss


# revision 21
# speedup vs baseline: 1.2160x; 1.2160x over previous
"""Trainium2 Bass kernel for ActionPrototypeLayer (retrieval_knn).

Problem math (B=16384, A=64 app types, P=32 prototypes each, D=1024):
  sel    = prototypes[ids]                       [B,P,D]
  scores = (pooled . sel) / sqrt(D)              [B,P]
  attn   = softmax(scores)                       [B,P]
  action = LayerNorm(attn @ sel + pooled)        [B,D]
  sim    = (norm(pooled) . norm(protos)) / TEMP  [B,A,P]
  loss   = max(mean(-log(pos/total)), 0)   pos/total from exp(sim)

Everything derives from one dense matmul G = pooled @ protos_flat^T
([B,1024] x [1024, 2048]).  protos_flat^T is pre-scaled by 1/||proto_j||
so exp(sim) comes straight out of PSUM via one ScalarE activation with a
per-partition scale (1/(||pooled_b||*TEMP)) and a fused row-sum (total).

Sharding: data-parallel over batch, 2048 rows/core on 8 cores; prototypes
replicated.  The host SORTS the batch by app_type_id before sharding (a
pure permutation, undone on unshard), so each 128-row tile's rows select
prototypes from only 1-2 contiguous 128-column chunks ("resident window").
The attention softmax, mask+reduce ops, and the second (attention-weighted)
matmul run only on that window instead of all 2048 columns.

Engine-placement notes:
  - All ScalarE activations stay inside ONE table set
    (natural_log_exp_and_others: exp/ln/square/copy/identity).  rsqrt and
    sqrt are computed as exp(+-0.5*ln(x)) to avoid ~1.3us table reloads.
  - pooled/protos are fed as bf16 from the host so transposed operand
    layouts come straight from DMA-transpose (2-byte dtype requirement);
    no PE transposes or PSUM bounce copies for them.
  - Squares for norms use VectorE scalar_tensor_tensor self-multiplies
    with fused accumulation (bf16 4x mode).

Per-core loss partial is summed on host during unshard.
"""

import numpy as np

import concourse.bass as bass
import concourse.tile as tile
from concourse import mybir
from concourse.bass import MemorySpace
from concourse.bass_utils import run_bass_kernel_spmd
from concourse.masks import make_identity

B, A, P, D = 16384, 64, 32, 1024
NCORES = 8
BL = B // NCORES              # rows per core
AP_ = A * P                   # 2048 flattened prototypes
NDC = D // 128                # 8 contraction chunks
NJC = AP_ // 128              # 16 prototype-row chunks
TEMP = 0.07
SQD = float(np.sqrt(D))
LN_EPS = 1e-5

F32 = mybir.dt.float32
BF16 = mybir.dt.bfloat16
F8 = mybir.dt.float8e4
AF = mybir.ActivationFunctionType
OP = mybir.AluOpType
AX = mybir.AxisListType


def _bcast_row(ap, parts, n):
    """AP reading a [1, n] DRAM view broadcast across `parts` partitions."""
    return bass.AP(tensor=ap.tensor, offset=ap.offset, ap=[[0, parts], [1, n]])


def build_nc(bl=BL, jsets=None, trivial_gamma=False, trivial_beta=False,
             split_waits=True):
    """jsets: per-batch-tile (jc_lo, jc_hi) resident prototype-chunk range
    (inclusive).  None -> dense (0, NJC-1) for every tile."""
    nbt = bl // 128
    if jsets is None:
        jsets = [(0, NJC - 1)] * nbt
    assert len(jsets) == nbt
    wmax = max(hi - lo + 1 for lo, hi in jsets) * 128

    nc = bass.Bass()

    pooled = nc.declare_dram_parameter("pooled_bf", [bl, D], BF16, isOutput=False)
    pooledT = nc.declare_dram_parameter("pooledT8", [D, bl], F8, isOutput=False)
    protos = nc.declare_dram_parameter("protos_bf", [AP_, D], BF16, isOutput=False)
    protosT = nc.declare_dram_parameter("protosT8", [D, AP_], F8, isOutput=False)
    idsf = nc.declare_dram_parameter("idsf", [bl, 1], F32, isOutput=False)
    jblock = nc.declare_dram_parameter("jblock", [1, AP_], F32, isOutput=False)
    gamma = nc.declare_dram_parameter("gamma", [1, D], F32, isOutput=False)
    beta = nc.declare_dram_parameter("beta", [1, D], F32, isOutput=False)
    act_out = nc.declare_dram_parameter("act_out", [bl, D], F32, isOutput=True)
    loss_out = nc.declare_dram_parameter("loss_out", [1, 1], F32, isOutput=True)

    norms_dram = nc.dram_tensor("norms_scratch", [2, AP_], F32)

    with tile.TileContext(nc) as tc:
        with (
            tc.tile_pool(name="consts", bufs=1) as consts,
            tc.tile_pool(name="protop", bufs=1) as protop,
            tc.tile_pool(name="work", bufs=3) as work,
            tc.tile_pool(name="big", bufs=3 if wmax <= 512 else 1) as big,
            tc.tile_pool(name="small", bufs=3) as small,
            tc.tile_pool(name="psA", bufs=2, space=MemorySpace.PSUM) as psA,
            tc.tile_pool(name="psB", bufs=2, space=MemorySpace.PSUM) as psB,
            tc.tile_pool(name="psT", bufs=2, space=MemorySpace.PSUM) as psT,
        ):
            # ---------------- constants ----------------
            ident = consts.tile([128, 128], BF16, tag="ident")
            make_identity(nc, ident[:])
            ident_f = consts.tile([128, 128], F32, tag="ident_f")
            make_identity(nc, ident_f[:])
            jblock_b = consts.tile([128, AP_], BF16, tag="jblock_b")
            nc.gpsimd.dma_start(out=jblock_b[:], in_=_bcast_row(jblock[:, :], 128, AP_))
            if not trivial_gamma:
                gamma_b = consts.tile([128, D], F32, tag="gamma_b")
                nc.sync.dma_start(out=gamma_b[:], in_=_bcast_row(gamma[:, :], 128, D))
            if not trivial_beta:
                beta_b = consts.tile([128, D], F32, tag="beta_b")
                nc.sync.dma_start(out=beta_b[:], in_=_bcast_row(beta[:, :], 128, D))
            ones_col = consts.tile([128, 1], F32, tag="ones_col")
            nc.vector.memset(ones_col[:], 1.0)
            ones_row = consts.tile([1, 128], BF16, tag="ones_row")
            nc.vector.memset(ones_row[:], 1.0)
            loss_acc = consts.tile([128, 1], F32, tag="loss_acc")
            nc.vector.memset(loss_acc[:], 0.0)
            b_eps10 = consts.tile([128, 1], F32, tag="b_eps10")
            nc.vector.memset(b_eps10[:], 1e-10)
            b_lneps = consts.tile([128, 1], F32, tag="b_lneps")
            nc.vector.memset(b_lneps[:], LN_EPS)
            b_nlt = consts.tile([128, 1], F32, tag="b_nlt")
            nc.vector.memset(b_nlt[:], float(-np.log(TEMP)))

            # ---------------- prototype / pooled prep ----------------
            # proto_nat[jc]: [128 j, 1024 d] bf16 (action-matmul rhs)
            # protoT[dc]:    [128 d, 2048 j] bf16 via DMA-transpose, then
            #                columns pre-scaled by 1/||proto_j||
            # poolT_all[dc]: [128 d, bl b] bf16 via DMA-transpose
            proto_nat = [protop.tile([128, D], BF16, tag=f"pn{jc}", name=f"pn{jc}")
                         for jc in range(NJC)]
            protoT8 = protop.tile([128, NDC, AP_], F8, tag="protoT8")
            poolT8 = protop.tile([128, NDC, bl], F8, tag="poolT8")
            invn_all = consts.tile([128, NJC], F32, tag="invn_all")
            norm_all = consts.tile([128, NJC], F32, tag="norm_all")

            for dc in range(NDC):
                nc.scalar.dma_start(
                    out=protoT8[:, dc, :], in_=protosT[dc * 128:(dc + 1) * 128, :])
                nc.scalar.dma_start(
                    out=poolT8[:, dc, :], in_=pooledT[dc * 128:(dc + 1) * 128, :])

            for jc in range(NJC):
                nc.sync.dma_start(out=proto_nat[jc][:],
                                  in_=protos[jc * 128:(jc + 1) * 128, :])
                scr = work.tile([128, D], BF16, tag="scr")
                sqs = small.tile([128, 1], F32, tag="sqs")
                nc.vector.scalar_tensor_tensor(
                    scr[:], proto_nat[jc][:], 1.0, proto_nat[jc][:],
                    op0=OP.bypass, op1=OP.mult, accum_out=sqs[:])
                lnz = small.tile([128, 1], F32, tag="lnz")
                nc.scalar.activation(lnz[:], sqs[:], AF.Ln)
                nc.scalar.activation(norm_all[:, jc:jc + 1], lnz[:], AF.Exp, scale=0.5)
                nc.scalar.activation(invn_all[:, jc:jc + 1], lnz[:], AF.Exp, scale=-0.5)

            # broadcast rows of 1/||p_j|| and ||p_j|| (both bf16):
            # transpose [128,16] -> [16,128] -> DRAM row -> partition-bcast
            invnp_b = consts.tile([128, AP_], BF16, tag="invnp_b")
            nprot_b = consts.tile([128, AP_], BF16, tag="nprot_b")
            for row, srct, dstb in ((0, invn_all, invnp_b), (1, norm_all, nprot_b)):
                tpn = psT.tile([16, 128], F32, tag="tp", name=f"tpn{row}")
                nc.tensor.transpose(tpn[:], srct[:], ident_f[:])
                t16 = consts.tile([16, 128], F32, tag=f"t16_{row}", name=f"t16_{row}")
                nc.vector.tensor_copy(t16[:], tpn[:])
                nd = norms_dram[row:row + 1, :]
                dst = bass.AP(tensor=nd.tensor, offset=nd.offset, ap=[[128, 16], [1, 128]])
                nc.sync.dma_start(out=dst, in_=t16[:])
                nc.gpsimd.dma_start(out=dstb[:], in_=_bcast_row(nd, 128, AP_))

            # pre-scale protoT8 columns by 1/||proto_j|| (in place)
            for dc in range(NDC):
                nc.vector.tensor_mul(protoT8[:, dc, :], protoT8[:, dc, :], invnp_b[:])

            # ---------------- per-batch-tile pipeline ----------------
            for bt in range(nbt):
                jlo, jhi = jsets[bt]
                nres = jhi - jlo + 1
                W = nres * 128
                jres = slice(jlo * 128, (jhi + 1) * 128)
                bsl = slice(bt * 128, (bt + 1) * 128)

                pool_nat = work.tile([128, D], BF16, tag="pool_nat")
                nc.sync.dma_start(out=pool_nat[:], in_=pooled[bsl, :])

                # ||pooled_b||^2 via DVE self-mult + accum;
                # escale = exp(-0.5*ln(sq) - ln(TEMP)) = 1/(||pooled||*TEMP)
                scr = work.tile([128, D], BF16, tag="scr")
                sqs = small.tile([128, 1], F32, tag="sqs")
                nc.vector.scalar_tensor_tensor(
                    scr[:], pool_nat[:], 1.0, pool_nat[:],
                    op0=OP.bypass, op1=OP.mult, accum_out=sqs[:])
                lnz = small.tile([128, 1], F32, tag="lnz")
                nc.scalar.activation(lnz[:], sqs[:], AF.Ln)
                escale = small.tile([128, 1], F32, tag="escale")
                nc.scalar.activation(escale[:], lnz[:], AF.Exp, scale=-0.5, bias=b_nlt[:])
                ids_c = small.tile([128, 1], F32, tag="ids_c")
                nc.sync.dma_start(out=ids_c[:], in_=idsf[bsl, :])

                # G matmul, fp8 DoubleRow (2 k-tiles per step): halves of 1024
                # E_c = exp(sim) straight from PSUM; per-half row-sums
                E_c = big.tile([128, AP_], BF16, tag="E_c")
                tsl = small.tile([128, 2], F32, tag="tsl")
                gps = []
                for hf in range(2):
                    gp = psA.tile([128, 1024], F32, tag="gp", name=f"gp{hf}")
                    gps.append(gp)
                    jsl = slice(hf * 1024, (hf + 1) * 1024)
                    for sub in range(2):
                        for q in range(4):
                            nc.tensor.matmul(
                                gp[:, sub * 512:(sub + 1) * 512],
                                poolT8[:, 2 * q:2 * q + 2, bsl],
                                protoT8[:, 2 * q:2 * q + 2,
                                        hf * 1024 + sub * 512:hf * 1024 + (sub + 1) * 512],
                                start=(q == 0),
                                stop=(q == 3),
                                perf_mode=mybir.MatmulPerfMode.DoubleRow,
                            )
                    nc.scalar.activation(E_c[:, jsl], gp[:], AF.Exp,
                                         scale=escale[:], accum_out=tsl[:, hf:hf + 1])
                total = small.tile([128, 1], F32, tag="total")
                nc.vector.tensor_add(total[:], tsl[:, 0:1], tsl[:, 1:2])

                # raw G on the resident window: G = gp * ||p_j||
                Graw = big.tile([128, wmax], F32, tag="Graw")
                for r in range(nres):
                    jc = jlo + r
                    hf = jc // 8
                    off = (jc % 8) * 128
                    nc.vector.tensor_mul(
                        Graw[:, r * 128:(r + 1) * 128],
                        gps[hf][:, off:off + 128],
                        nprot_b[:, jc * 128:(jc + 1) * 128],
                    )
                # attention numerators on the window
                E_a = big.tile([128, wmax], F32, tag="E_a")
                nc.scalar.activation(E_a[:, :W], Graw[:, :W], AF.Exp, scale=1.0 / SQD)

                # pos = sum over own block of E_c (mask fused)
                Ecm = big.tile([128, wmax], BF16, tag="Ecm")
                pos = small.tile([128, 1], F32, tag="pos")
                nc.vector.scalar_tensor_tensor(
                    Ecm[:, :W], jblock_b[:, jres], ids_c[:], E_c[:, jres],
                    op0=OP.is_equal, op1=OP.mult, accum_out=pos[:],
                )
                # attention: masked numerators + denominator
                E_am = big.tile([128, wmax], BF16, tag="E_am")
                den = small.tile([128, 1], F32, tag="den")
                nc.vector.scalar_tensor_tensor(
                    E_am[:, :W], jblock_b[:, jres], ids_c[:], E_a[:, :W],
                    op0=OP.is_equal, op1=OP.mult, accum_out=den[:],
                )

                # loss_acc += ln(total + 1e-10) - ln(pos)
                lt = small.tile([128, 1], F32, tag="lt")
                nc.scalar.activation(lt[:], total[:], AF.Ln, bias=b_eps10[:])
                lp = small.tile([128, 1], F32, tag="lp")
                nc.scalar.activation(lp[:], pos[:], AF.Ln)
                dlt = small.tile([128, 1], F32, tag="dlt")
                nc.vector.tensor_sub(dlt[:], lt[:], lp[:])
                nc.vector.tensor_add(loss_acc[:], loss_acc[:], dlt[:])

                # transpose E_am -> lhsT chunks
                E_amT = big.tile([128, wmax], BF16, tag="E_amT")
                for g in range((nres + 3) // 4):
                    kn = min(4, nres - g * 4)
                    tp = psT.tile([128, 4, 128], BF16, tag="tp")
                    for k in range(kn):
                        r = g * 4 + k
                        nc.tensor.transpose(tp[:, k], E_am[:, r * 128:(r + 1) * 128], ident[:])
                    nc.vector.tensor_copy(
                        E_amT[:, g * 512:g * 512 + kn * 128].rearrange("p (k f) -> p k f", k=kn),
                        tp[:, 0:kn],
                    )

                # action matmul over resident chunks only;
                # act1 = U/den + pooled with fused row-sum (LayerNorm mean)
                recip_den = small.tile([128, 1], F32, tag="recip_den")
                nc.vector.reciprocal(recip_den[:], den[:])
                act1 = work.tile([128, D], F32, tag="act1")
                s1p = small.tile([128, 2], F32, tag="s1p")
                for dsl in range(2):
                    up = psB.tile([128, 512], F32, tag="up")
                    dslc = slice(dsl * 512, (dsl + 1) * 512)
                    for r in range(nres):
                        jc = jlo + r
                        nc.tensor.matmul(
                            up[:],
                            E_amT[:, r * 128:(r + 1) * 128],
                            proto_nat[jc][:, dslc],
                            start=(r == 0),
                            stop=(r == nres - 1),
                        )
                    nc.vector.scalar_tensor_tensor(
                        act1[:, dslc], up[:], recip_den[:], pool_nat[:, dslc],
                        op0=OP.mult, op1=OP.add, accum_out=s1p[:, dsl:dsl + 1],
                    )

                # LayerNorm(act1) * gamma + beta
                sum1 = small.tile([128, 1], F32, tag="sum1")
                nc.vector.tensor_add(sum1[:], s1p[:, 0:1], s1p[:, 1:2])
                negmu = small.tile([128, 1], F32, tag="negmu")
                nc.vector.tensor_scalar_mul(negmu[:], sum1[:], -1.0 / D)
                xc = work.tile([128, D], F32, tag="xc")
                nc.vector.tensor_scalar_add(xc[:], act1[:], negmu[:])
                scr2 = work.tile([128, D], BF16, tag="scr")
                vs = small.tile([128, 1], F32, tag="vs")
                nc.scalar.activation(scr2[:], xc[:], AF.Square, accum_out=vs[:])
                # rstd = exp(-0.5*ln(vs/D + eps))
                lnv = small.tile([128, 1], F32, tag="lnv")
                nc.scalar.activation(lnv[:], vs[:], AF.Ln, scale=1.0 / D, bias=b_lneps[:])
                rstd = small.tile([128, 1], F32, tag="rstd")
                nc.scalar.activation(rstd[:], lnv[:], AF.Exp, scale=-0.5)
                outt = work.tile([128, D], F32, tag="outt")
                if trivial_gamma:
                    nc.vector.tensor_scalar_mul(outt[:], xc[:], rstd[:])
                else:
                    nc.vector.scalar_tensor_tensor(
                        outt[:], xc[:], rstd[:], gamma_b[:],
                        op0=OP.mult, op1=OP.mult,
                    )
                if not trivial_beta:
                    nc.vector.tensor_add(outt[:], outt[:], beta_b[:])
                nc.scalar.dma_start(out=act_out[bsl, :], in_=outt[:])

            # ---------------- loss partial: partition-sum ----------------
            lps = psT.tile([1, 1], F32, tag="tp")
            nc.tensor.matmul(lps[:], loss_acc[:], ones_col[:], start=True, stop=True)
            lsb = small.tile([1, 1], F32, tag="lsb")
            nc.vector.tensor_copy(lsb[:], lps[:])
            nc.sync.dma_start(out=loss_out[:, :], in_=lsb[:])

    if split_waits:
        _split_multi_waits(nc)
    return nc


def _split_multi_waits(nc, max_cmds=2):
    """This walrus build allows at most ~2 sync commands (waits+updates) per
    instruction.  Tile emits up to 3+ waits on fan-in instructions; hoist the
    excess waits onto single-wait ENGINE_NOPs placed just before, on the same
    engine (same blocking semantics, engine streams run in program order)."""
    for fn in nc.m.functions:
        for blk in fn.blocks:
            new = []
            for inst in blk.instructions:
                si = getattr(inst, "sync_info", None)
                waits = list(si.on_wait) if si is not None and si.on_wait else []
                ups = list(si.on_update) if si is not None and si.on_update else []
                budget = min(1, max(0, max_cmds - len(ups)))
                if len(waits) > budget:
                    nkeep = budget
                    extra, kept = waits[:len(waits) - nkeep], waits[len(waits) - nkeep:]
                    for w in extra:
                        nop = mybir.InstEventSemaphore(
                            name=nc.get_next_instruction_name(),
                            engine=inst.engine,
                            ins=[],
                            outs=[],
                        )
                        nop.sync_info = mybir.SyncInfo(on_wait=[w], on_update=[])
                        new.append(nop)
                    inst.sync_info = mybir.SyncInfo(on_wait=kept, on_update=ups)
                new.append(inst)
            blk.instructions = new


_NC_CACHE = {}


def _get_nc(bl=BL, jsets=None, trivial_gamma=False, trivial_beta=False):
    key = (bl, tuple(jsets) if jsets is not None else None, trivial_gamma, trivial_beta)
    if key not in _NC_CACHE:
        _NC_CACHE[key] = build_nc(bl, jsets, trivial_gamma, trivial_beta)
    return _NC_CACHE[key]


def plan_shards(app_type_ids, ncores=NCORES, bl=BL):
    """Sort batch by id, then deal the 128-row sorted tiles round-robin to
    cores (core = g % ncores, slot = g // ncores).  Tile-slot s covers nearly
    the same id range on every core, so ONE SPMD graph (with the per-slot
    union of resident chunk ranges) serves all cores."""
    ids = np.asarray(app_type_ids).astype(np.int64).reshape(-1)
    base = np.argsort(ids, kind="stable")
    ids_sorted = ids[base]
    ngt = len(ids) // 128
    nslots = ngt // ncores
    order = []
    for c in range(ncores):
        for s in range(nslots):
            g = s * ncores + c
            order.append(base[128 * g:128 * (g + 1)])
    perm = np.concatenate(order)
    jsets = []
    for s in range(nslots):
        lo_id = int(ids_sorted[128 * (s * ncores)])
        hi_id = int(ids_sorted[128 * (s * ncores + ncores - 1) + 127])
        jsets.append((lo_id * P // 128, hi_id * P // 128))
    return perm, jsets


def make_in_maps(pooled_output, app_type_ids, prototypes, ln_gamma, ln_beta,
                 perm=None, ncores=NCORES, bl=BL):
    bf16 = mybir.dt.np(BF16)
    f8 = mybir.dt.np(F8)
    protoflat = np.asarray(prototypes, dtype=np.float32).reshape(AP_, D).astype(bf16)
    protoT8 = np.ascontiguousarray(protoflat.T).astype(f8)
    jblock = (np.arange(AP_, dtype=np.int64) // P).astype(np.float32).reshape(1, AP_)
    gamma = np.asarray(ln_gamma, dtype=np.float32).reshape(1, D)
    beta = np.asarray(ln_beta, dtype=np.float32).reshape(1, D)
    pooled_bf = np.asarray(pooled_output, dtype=np.float32).astype(bf16)
    idsf = np.asarray(app_type_ids).astype(np.float32).reshape(-1, 1)
    if perm is not None:
        pooled_bf = pooled_bf[perm]
        idsf = idsf[perm]
    in_maps = []
    for c in range(ncores):
        sl = slice(c * bl, (c + 1) * bl)
        in_maps.append({
            "pooled_bf": np.ascontiguousarray(pooled_bf[sl]),
            "pooledT8": np.ascontiguousarray(pooled_bf[sl].T).astype(f8),
            "protos_bf": protoflat,
            "protosT8": protoT8,
            "idsf": np.ascontiguousarray(idsf[sl]),
            "jblock": jblock,
            "gamma": gamma,
            "beta": beta,
        })
    return in_maps


def _prep(pooled_output, app_type_ids, prototypes, ln_gamma, ln_beta):
    perm, jsets = plan_shards(app_type_ids)
    tg = bool(np.all(np.asarray(ln_gamma) == 1.0))
    tb = bool(np.all(np.asarray(ln_beta) == 0.0))
    nc = _get_nc(BL, jsets, tg, tb)
    in_maps = make_in_maps(pooled_output, app_type_ids, prototypes,
                           ln_gamma, ln_beta, perm=perm)
    return nc, in_maps, perm


def kernel(pooled_output, app_type_ids, prototypes, ln_gamma, ln_beta):
    nc, in_maps, perm = _prep(pooled_output, app_type_ids, prototypes,
                              ln_gamma, ln_beta)
    res = run_bass_kernel_spmd(nc, in_maps, core_ids=list(range(NCORES)))
    action_sorted = np.concatenate([r["act_out"] for r in res.results], axis=0)
    action = np.empty_like(action_sorted)
    action[perm] = action_sorted
    loss_sum = sum(float(r["loss_out"][0, 0]) for r in res.results)
    loss = np.float32(max(loss_sum / B, 0.0))
    return action.astype(np.float32), loss


# revision 22
# speedup vs baseline: 1.2882x; 1.0593x over previous
"""Trainium2 Bass kernel for ActionPrototypeLayer (retrieval_knn).

Problem math (B=16384, A=64 app types, P=32 prototypes each, D=1024):
  sel    = prototypes[ids]                       [B,P,D]
  scores = (pooled . sel) / sqrt(D)              [B,P]
  attn   = softmax(scores)                       [B,P]
  action = LayerNorm(attn @ sel + pooled)        [B,D]
  sim    = (norm(pooled) . norm(protos)) / TEMP  [B,A,P]
  loss   = max(mean(-log(pos/total)), 0)   pos/total from exp(sim)

Everything derives from one dense matmul G = pooled @ protos_flat^T
([B,1024] x [1024, 2048]).  protos_flat^T is pre-scaled by 1/||proto_j||
so exp(sim) comes straight out of PSUM via one ScalarE activation with a
per-partition scale (1/(||pooled_b||*TEMP)) and a fused row-sum (total).

Sharding: data-parallel over batch, 2048 rows/core on 8 cores; prototypes
replicated.  The host SORTS the batch by app_type_id before sharding (a
pure permutation, undone on unshard), so each 128-row tile's rows select
prototypes from only 1-2 contiguous 128-column chunks ("resident window").
The attention softmax, mask+reduce ops, and the second (attention-weighted)
matmul run only on that window instead of all 2048 columns.

Engine-placement notes:
  - All ScalarE activations stay inside ONE table set
    (natural_log_exp_and_others: exp/ln/square/copy/identity).  rsqrt and
    sqrt are computed as exp(+-0.5*ln(x)) to avoid ~1.3us table reloads.
  - pooled/protos are fed as bf16 from the host so transposed operand
    layouts come straight from DMA-transpose (2-byte dtype requirement);
    no PE transposes or PSUM bounce copies for them.
  - Squares for norms use VectorE scalar_tensor_tensor self-multiplies
    with fused accumulation (bf16 4x mode).

Per-core loss partial is summed on host during unshard.
"""

import numpy as np

import concourse.bass as bass
import concourse.tile as tile
from concourse import mybir
from concourse.bass import MemorySpace
from concourse.bass_utils import run_bass_kernel_spmd
from concourse.masks import make_identity

B, A, P, D = 16384, 64, 32, 1024
NCORES = 8
BL = B // NCORES              # rows per core
AP_ = A * P                   # 2048 flattened prototypes
NDC = D // 128                # 8 contraction chunks
NJC = AP_ // 128              # 16 prototype-row chunks
TEMP = 0.07
SQD = float(np.sqrt(D))
LN_EPS = 1e-5

F32 = mybir.dt.float32
BF16 = mybir.dt.bfloat16
F8 = mybir.dt.float8e4
AF = mybir.ActivationFunctionType
OP = mybir.AluOpType
AX = mybir.AxisListType


def _bcast_row(ap, parts, n):
    """AP reading a [1, n] DRAM view broadcast across `parts` partitions."""
    return bass.AP(tensor=ap.tensor, offset=ap.offset, ap=[[0, parts], [1, n]])


def build_nc(bl=BL, jsets=None, trivial_gamma=False, trivial_beta=False,
             split_waits=True):
    """jsets: per-batch-tile (jc_lo, jc_hi) resident prototype-chunk range
    (inclusive).  None -> dense (0, NJC-1) for every tile."""
    nbt = bl // 128
    if jsets is None:
        jsets = [(0, NJC - 1)] * nbt
    assert len(jsets) == nbt
    wmax = max(hi - lo + 1 for lo, hi in jsets) * 128

    nc = bass.Bass()

    pooled = nc.declare_dram_parameter("pooled_bf", [bl, D], BF16, isOutput=False)
    pooledT = nc.declare_dram_parameter("pooledT8", [D, bl], F8, isOutput=False)
    protos = nc.declare_dram_parameter("protos_bf", [AP_, D], BF16, isOutput=False)
    protosT = nc.declare_dram_parameter("protosT8", [D, AP_], F8, isOutput=False)
    idsf = nc.declare_dram_parameter("idsf", [bl, 1], F32, isOutput=False)
    jblock = nc.declare_dram_parameter("jblock", [1, AP_], F32, isOutput=False)
    gamma = nc.declare_dram_parameter("gamma", [1, D], F32, isOutput=False)
    beta = nc.declare_dram_parameter("beta", [1, D], F32, isOutput=False)
    act_out = nc.declare_dram_parameter("act_out", [bl, D], F32, isOutput=True)
    loss_out = nc.declare_dram_parameter("loss_out", [1, 1], F32, isOutput=True)

    norms_dram = nc.dram_tensor("norms_scratch", [2, AP_], F32)

    with tile.TileContext(nc) as tc:
        with (
            tc.tile_pool(name="consts", bufs=1) as consts,
            tc.tile_pool(name="protop", bufs=1) as protop,
            tc.tile_pool(name="work", bufs=4) as work,
            tc.tile_pool(name="big", bufs=4 if wmax <= 512 else 1) as big,
            tc.tile_pool(name="small", bufs=6) as small,
            tc.tile_pool(name="psA", bufs=2, space=MemorySpace.PSUM) as psA,
            tc.tile_pool(name="psB", bufs=2, space=MemorySpace.PSUM) as psB,
            tc.tile_pool(name="psT", bufs=2, space=MemorySpace.PSUM) as psT,
        ):
            # ---------------- constants ----------------
            ident = consts.tile([128, 128], BF16, tag="ident")
            make_identity(nc, ident[:])
            ident_f = consts.tile([128, 128], F32, tag="ident_f")
            make_identity(nc, ident_f[:])
            jblock_b = consts.tile([128, AP_], BF16, tag="jblock_b")
            nc.gpsimd.dma_start(out=jblock_b[:], in_=_bcast_row(jblock[:, :], 128, AP_))
            if not trivial_gamma:
                gamma_b = consts.tile([128, D], F32, tag="gamma_b")
                nc.sync.dma_start(out=gamma_b[:], in_=_bcast_row(gamma[:, :], 128, D))
            if not trivial_beta:
                beta_b = consts.tile([128, D], F32, tag="beta_b")
                nc.sync.dma_start(out=beta_b[:], in_=_bcast_row(beta[:, :], 128, D))
            ones_col = consts.tile([128, 1], F32, tag="ones_col")
            nc.vector.memset(ones_col[:], 1.0)
            ones_row = consts.tile([1, 128], BF16, tag="ones_row")
            nc.vector.memset(ones_row[:], 1.0)
            loss_acc = consts.tile([128, 1], F32, tag="loss_acc")
            nc.vector.memset(loss_acc[:], 0.0)
            b_eps10 = consts.tile([128, 1], F32, tag="b_eps10")
            nc.vector.memset(b_eps10[:], 1e-10)
            b_lneps = consts.tile([128, 1], F32, tag="b_lneps")
            nc.vector.memset(b_lneps[:], LN_EPS)
            b_nlt = consts.tile([128, 1], F32, tag="b_nlt")
            nc.vector.memset(b_nlt[:], float(-np.log(TEMP)))

            # ---------------- prototype / pooled prep ----------------
            # proto_nat[jc]: [128 j, 1024 d] bf16 (action-matmul rhs)
            # protoT[dc]:    [128 d, 2048 j] bf16 via DMA-transpose, then
            #                columns pre-scaled by 1/||proto_j||
            # poolT_all[dc]: [128 d, bl b] bf16 via DMA-transpose
            proto_nat = [protop.tile([128, D], BF16, tag=f"pn{jc}", name=f"pn{jc}")
                         for jc in range(NJC)]
            protoT8 = protop.tile([128, NDC, AP_], F8, tag="protoT8")
            poolT8 = protop.tile([128, NDC, bl], F8, tag="poolT8")
            invn_all = consts.tile([128, NJC], F32, tag="invn_all")
            norm_all = consts.tile([128, NJC], F32, tag="norm_all")

            for dc in range(NDC):
                nc.scalar.dma_start(
                    out=protoT8[:, dc, :], in_=protosT[dc * 128:(dc + 1) * 128, :])
                nc.scalar.dma_start(
                    out=poolT8[:, dc, :], in_=pooledT[dc * 128:(dc + 1) * 128, :])

            for jc in range(NJC):
                nc.sync.dma_start(out=proto_nat[jc][:],
                                  in_=protos[jc * 128:(jc + 1) * 128, :])
                scr = work.tile([128, D], BF16, tag="scr")
                sqs = small.tile([128, 1], F32, tag="sqs")
                if jc % 2 == 0:
                    nc.vector.scalar_tensor_tensor(
                        scr[:], proto_nat[jc][:], 1.0, proto_nat[jc][:],
                        op0=OP.bypass, op1=OP.mult, accum_out=sqs[:])
                else:
                    nc.scalar.activation(scr[:], proto_nat[jc][:], AF.Square,
                                         accum_out=sqs[:])
                lnz = small.tile([128, 1], F32, tag="lnz")
                nc.scalar.activation(lnz[:], sqs[:], AF.Ln)
                nc.scalar.activation(norm_all[:, jc:jc + 1], lnz[:], AF.Exp, scale=0.5)
                nc.scalar.activation(invn_all[:, jc:jc + 1], lnz[:], AF.Exp, scale=-0.5)

            # broadcast rows of 1/||p_j|| and ||p_j|| (both bf16):
            # transpose [128,16] -> [16,128] -> DRAM row -> partition-bcast
            invnp_b = consts.tile([128, AP_], BF16, tag="invnp_b")
            nprot_b = consts.tile([128, AP_], BF16, tag="nprot_b")
            for row, srct, dstb in ((0, invn_all, invnp_b), (1, norm_all, nprot_b)):
                tpn = psT.tile([16, 128], F32, tag="tp", name=f"tpn{row}")
                nc.tensor.transpose(tpn[:], srct[:], ident_f[:])
                t16 = consts.tile([16, 128], F32, tag=f"t16_{row}", name=f"t16_{row}")
                nc.vector.tensor_copy(t16[:], tpn[:])
                nd = norms_dram[row:row + 1, :]
                dst = bass.AP(tensor=nd.tensor, offset=nd.offset, ap=[[128, 16], [1, 128]])
                nc.sync.dma_start(out=dst, in_=t16[:])
                nc.gpsimd.dma_start(out=dstb[:], in_=_bcast_row(nd, 128, AP_))

            # pre-scale protoT8 columns by 1/||proto_j|| (in place)
            for dc in range(NDC):
                nc.vector.tensor_mul(protoT8[:, dc, :], protoT8[:, dc, :], invnp_b[:])

            # ---------------- per-batch-tile pipeline ----------------
            for bt in range(nbt):
                jlo, jhi = jsets[bt]
                nres = jhi - jlo + 1
                W = nres * 128
                jres = slice(jlo * 128, (jhi + 1) * 128)
                bsl = slice(bt * 128, (bt + 1) * 128)

                pool_nat = work.tile([128, D], BF16, tag="pool_nat")
                nc.sync.dma_start(out=pool_nat[:], in_=pooled[bsl, :])

                # ||pooled_b||^2 via DVE self-mult + accum;
                # escale = exp(-0.5*ln(sq) - ln(TEMP)) = 1/(||pooled||*TEMP)
                scr = work.tile([128, D], BF16, tag="scr")
                sqs = small.tile([128, 1], F32, tag="sqs")
                nc.vector.scalar_tensor_tensor(
                    scr[:], pool_nat[:], 1.0, pool_nat[:],
                    op0=OP.bypass, op1=OP.mult, accum_out=sqs[:])
                lnz = small.tile([128, 1], F32, tag="lnz")
                nc.scalar.activation(lnz[:], sqs[:], AF.Ln)
                escale = small.tile([128, 1], F32, tag="escale")
                nc.scalar.activation(escale[:], lnz[:], AF.Exp, scale=-0.5, bias=b_nlt[:])
                ids_c = small.tile([128, 1], F32, tag="ids_c")
                nc.sync.dma_start(out=ids_c[:], in_=idsf[bsl, :])

                # G matmul, fp8 DoubleRow (2 k-tiles per step): halves of 1024
                # E_c = exp(sim) straight from PSUM; per-half row-sums
                E_c = big.tile([128, AP_], BF16, tag="E_c")
                tsl = small.tile([128, 2], F32, tag="tsl")
                gps = []
                for hf in range(2):
                    gp = psA.tile([128, 1024], F32, tag="gp", name=f"gp{hf}")
                    gps.append(gp)
                    jsl = slice(hf * 1024, (hf + 1) * 1024)
                    for sub in range(2):
                        for q in range(4):
                            nc.tensor.matmul(
                                gp[:, sub * 512:(sub + 1) * 512],
                                poolT8[:, 2 * q:2 * q + 2, bsl],
                                protoT8[:, 2 * q:2 * q + 2,
                                        hf * 1024 + sub * 512:hf * 1024 + (sub + 1) * 512],
                                start=(q == 0),
                                stop=(q == 3),
                                perf_mode=mybir.MatmulPerfMode.DoubleRow,
                            )
                    nc.scalar.activation(E_c[:, jsl], gp[:], AF.Exp,
                                         scale=escale[:], accum_out=tsl[:, hf:hf + 1])
                total = small.tile([128, 1], F32, tag="total")
                nc.vector.tensor_add(total[:], tsl[:, 0:1], tsl[:, 1:2])

                # raw G on the resident window: G = gp * ||p_j||
                Graw = big.tile([128, wmax], F32, tag="Graw")
                for r in range(nres):
                    jc = jlo + r
                    hf = jc // 8
                    off = (jc % 8) * 128
                    nc.vector.tensor_mul(
                        Graw[:, r * 128:(r + 1) * 128],
                        gps[hf][:, off:off + 128],
                        nprot_b[:, jc * 128:(jc + 1) * 128],
                    )
                # attention numerators on the window
                E_a = big.tile([128, wmax], F32, tag="E_a")
                nc.scalar.activation(E_a[:, :W], Graw[:, :W], AF.Exp, scale=1.0 / SQD)

                # pos = sum over own block of E_c (mask fused)
                Ecm = big.tile([128, wmax], BF16, tag="Ecm")
                pos = small.tile([128, 1], F32, tag="pos")
                nc.vector.scalar_tensor_tensor(
                    Ecm[:, :W], jblock_b[:, jres], ids_c[:], E_c[:, jres],
                    op0=OP.is_equal, op1=OP.mult, accum_out=pos[:],
                )
                # attention: masked numerators + denominator
                E_am = big.tile([128, wmax], BF16, tag="E_am")
                den = small.tile([128, 1], F32, tag="den")
                nc.vector.scalar_tensor_tensor(
                    E_am[:, :W], jblock_b[:, jres], ids_c[:], E_a[:, :W],
                    op0=OP.is_equal, op1=OP.mult, accum_out=den[:],
                )

                # loss_acc += ln(total + 1e-10) - ln(pos)
                lt = small.tile([128, 1], F32, tag="lt")
                nc.scalar.activation(lt[:], total[:], AF.Ln, bias=b_eps10[:])
                lp = small.tile([128, 1], F32, tag="lp")
                nc.scalar.activation(lp[:], pos[:], AF.Ln)
                dlt = small.tile([128, 1], F32, tag="dlt")
                nc.vector.tensor_sub(dlt[:], lt[:], lp[:])
                nc.vector.tensor_add(loss_acc[:], loss_acc[:], dlt[:])

                # transpose E_am -> lhsT chunks
                E_amT = big.tile([128, wmax], BF16, tag="E_amT")
                for g in range((nres + 3) // 4):
                    kn = min(4, nres - g * 4)
                    tp = psT.tile([128, 4, 128], BF16, tag="tp")
                    for k in range(kn):
                        r = g * 4 + k
                        nc.tensor.transpose(tp[:, k], E_am[:, r * 128:(r + 1) * 128], ident[:])
                    nc.vector.tensor_copy(
                        E_amT[:, g * 512:g * 512 + kn * 128].rearrange("p (k f) -> p k f", k=kn),
                        tp[:, 0:kn],
                    )

                # action matmul over resident chunks only;
                # act1 = U/den + pooled with fused row-sum (LayerNorm mean)
                recip_den = small.tile([128, 1], F32, tag="recip_den")
                nc.vector.reciprocal(recip_den[:], den[:])
                act1 = work.tile([128, D], F32, tag="act1")
                s1p = small.tile([128, 2], F32, tag="s1p")
                for dsl in range(2):
                    up = psB.tile([128, 512], F32, tag="up")
                    dslc = slice(dsl * 512, (dsl + 1) * 512)
                    for r in range(nres):
                        jc = jlo + r
                        nc.tensor.matmul(
                            up[:],
                            E_amT[:, r * 128:(r + 1) * 128],
                            proto_nat[jc][:, dslc],
                            start=(r == 0),
                            stop=(r == nres - 1),
                        )
                    nc.vector.scalar_tensor_tensor(
                        act1[:, dslc], up[:], recip_den[:], pool_nat[:, dslc],
                        op0=OP.mult, op1=OP.add, accum_out=s1p[:, dsl:dsl + 1],
                    )

                # LayerNorm via E[x^2]-mu^2 (single pass over act1):
                #   mu = (s1p0+s1p1)/D;  var = sumsq/D - mu^2
                #   out = act1*rstd - mu*rstd   (one fused two-scalar op)
                scr2 = work.tile([128, D], BF16, tag="scr")
                vs = small.tile([128, 1], F32, tag="vs")
                nc.scalar.activation(scr2[:], act1[:], AF.Square, accum_out=vs[:])
                s1 = small.tile([128, 1], F32, tag="s1")
                nc.vector.tensor_add(s1[:], s1p[:, 0:1], s1p[:, 1:2])
                mu = small.tile([128, 1], F32, tag="mu")
                nc.vector.tensor_scalar_mul(mu[:], s1[:], 1.0 / D)
                musq = small.tile([128, 1], F32, tag="musq")
                nc.vector.tensor_mul(musq[:], mu[:], mu[:])
                var = small.tile([128, 1], F32, tag="var")
                nc.vector.scalar_tensor_tensor(
                    var[:], vs[:], 1.0 / D, musq[:],
                    op0=OP.mult, op1=OP.subtract)
                lnv = small.tile([128, 1], F32, tag="lnv")
                nc.scalar.activation(lnv[:], var[:], AF.Ln, bias=b_lneps[:])
                rstd = small.tile([128, 1], F32, tag="rstd")
                nc.scalar.activation(rstd[:], lnv[:], AF.Exp, scale=-0.5)
                murstd = small.tile([128, 1], F32, tag="murstd")
                nc.vector.tensor_mul(murstd[:], mu[:], rstd[:])
                outt = work.tile([128, D], F32, tag="outt")
                if trivial_gamma and trivial_beta:
                    nc.vector.tensor_scalar(
                        outt[:], act1[:], rstd[:], murstd[:],
                        op0=OP.mult, op1=OP.subtract)
                else:
                    xn = work.tile([128, D], F32, tag="xn")
                    nc.vector.tensor_scalar(
                        xn[:], act1[:], rstd[:], murstd[:],
                        op0=OP.mult, op1=OP.subtract)
                    if not trivial_gamma:
                        nc.vector.tensor_mul(outt[:], xn[:], gamma_b[:])
                    else:
                        nc.vector.tensor_copy(outt[:], xn[:])
                    if not trivial_beta:
                        nc.vector.tensor_add(outt[:], outt[:], beta_b[:])
                nc.scalar.dma_start(out=act_out[bsl, :], in_=outt[:])

            # ---------------- loss partial: partition-sum ----------------
            lps = psT.tile([1, 1], F32, tag="tp")
            nc.tensor.matmul(lps[:], loss_acc[:], ones_col[:], start=True, stop=True)
            lsb = small.tile([1, 1], F32, tag="lsb")
            nc.vector.tensor_copy(lsb[:], lps[:])
            nc.sync.dma_start(out=loss_out[:, :], in_=lsb[:])

    if split_waits:
        _split_multi_waits(nc)
    return nc


def _split_multi_waits(nc, max_cmds=2):
    """This walrus build allows at most ~2 sync commands (waits+updates) per
    instruction.  Tile emits up to 3+ waits on fan-in instructions; hoist the
    excess waits onto single-wait ENGINE_NOPs placed just before, on the same
    engine (same blocking semantics, engine streams run in program order)."""
    for fn in nc.m.functions:
        for blk in fn.blocks:
            new = []
            for inst in blk.instructions:
                si = getattr(inst, "sync_info", None)
                waits = list(si.on_wait) if si is not None and si.on_wait else []
                ups = list(si.on_update) if si is not None and si.on_update else []
                budget = min(1, max(0, max_cmds - len(ups)))
                if len(waits) > budget:
                    nkeep = budget
                    extra, kept = waits[:len(waits) - nkeep], waits[len(waits) - nkeep:]
                    for w in extra:
                        nop = mybir.InstEventSemaphore(
                            name=nc.get_next_instruction_name(),
                            engine=inst.engine,
                            ins=[],
                            outs=[],
                        )
                        nop.sync_info = mybir.SyncInfo(on_wait=[w], on_update=[])
                        new.append(nop)
                    inst.sync_info = mybir.SyncInfo(on_wait=kept, on_update=ups)
                new.append(inst)
            blk.instructions = new


_NC_CACHE = {}


def _get_nc(bl=BL, jsets=None, trivial_gamma=False, trivial_beta=False):
    key = (bl, tuple(jsets) if jsets is not None else None, trivial_gamma, trivial_beta)
    if key not in _NC_CACHE:
        _NC_CACHE[key] = build_nc(bl, jsets, trivial_gamma, trivial_beta)
    return _NC_CACHE[key]


def plan_shards(app_type_ids, ncores=NCORES, bl=BL):
    """Sort batch by id, then deal the 128-row sorted tiles round-robin to
    cores (core = g % ncores, slot = g // ncores).  Tile-slot s covers nearly
    the same id range on every core, so ONE SPMD graph (with the per-slot
    union of resident chunk ranges) serves all cores."""
    ids = np.asarray(app_type_ids).astype(np.int64).reshape(-1)
    base = np.argsort(ids, kind="stable")
    ids_sorted = ids[base]
    ngt = len(ids) // 128
    nslots = ngt // ncores
    order = []
    for c in range(ncores):
        for s in range(nslots):
            g = s * ncores + c
            order.append(base[128 * g:128 * (g + 1)])
    perm = np.concatenate(order)
    jsets = []
    for s in range(nslots):
        lo_id = int(ids_sorted[128 * (s * ncores)])
        hi_id = int(ids_sorted[128 * (s * ncores + ncores - 1) + 127])
        jsets.append((lo_id * P // 128, hi_id * P // 128))
    return perm, jsets


def make_in_maps(pooled_output, app_type_ids, prototypes, ln_gamma, ln_beta,
                 perm=None, ncores=NCORES, bl=BL):
    bf16 = mybir.dt.np(BF16)
    f8 = mybir.dt.np(F8)
    protoflat = np.asarray(prototypes, dtype=np.float32).reshape(AP_, D).astype(bf16)
    protoT8 = np.ascontiguousarray(protoflat.T).astype(f8)
    jblock = (np.arange(AP_, dtype=np.int64) // P).astype(np.float32).reshape(1, AP_)
    gamma = np.asarray(ln_gamma, dtype=np.float32).reshape(1, D)
    beta = np.asarray(ln_beta, dtype=np.float32).reshape(1, D)
    pooled_bf = np.asarray(pooled_output, dtype=np.float32).astype(bf16)
    idsf = np.asarray(app_type_ids).astype(np.float32).reshape(-1, 1)
    if perm is not None:
        pooled_bf = pooled_bf[perm]
        idsf = idsf[perm]
    in_maps = []
    for c in range(ncores):
        sl = slice(c * bl, (c + 1) * bl)
        in_maps.append({
            "pooled_bf": np.ascontiguousarray(pooled_bf[sl]),
            "pooledT8": np.ascontiguousarray(pooled_bf[sl].T).astype(f8),
            "protos_bf": protoflat,
            "protosT8": protoT8,
            "idsf": np.ascontiguousarray(idsf[sl]),
            "jblock": jblock,
            "gamma": gamma,
            "beta": beta,
        })
    return in_maps


def _prep(pooled_output, app_type_ids, prototypes, ln_gamma, ln_beta):
    perm, jsets = plan_shards(app_type_ids)
    tg = bool(np.all(np.asarray(ln_gamma) == 1.0))
    tb = bool(np.all(np.asarray(ln_beta) == 0.0))
    nc = _get_nc(BL, jsets, tg, tb)
    in_maps = make_in_maps(pooled_output, app_type_ids, prototypes,
                           ln_gamma, ln_beta, perm=perm)
    return nc, in_maps, perm


def kernel(pooled_output, app_type_ids, prototypes, ln_gamma, ln_beta):
    nc, in_maps, perm = _prep(pooled_output, app_type_ids, prototypes,
                              ln_gamma, ln_beta)
    res = run_bass_kernel_spmd(nc, in_maps, core_ids=list(range(NCORES)))
    action_sorted = np.concatenate([r["act_out"] for r in res.results], axis=0)
    action = np.empty_like(action_sorted)
    action[perm] = action_sorted
    loss_sum = sum(float(r["loss_out"][0, 0]) for r in res.results)
    loss = np.float32(max(loss_sum / B, 0.0))
    return action.astype(np.float32), loss


# revision 24
# speedup vs baseline: 1.7530x; 1.3609x over previous
"""Trainium2 Bass kernel for ActionPrototypeLayer (retrieval_knn).

Problem math (B=16384, A=64 app types, P=32 prototypes each, D=1024):
  sel    = prototypes[ids]                       [B,P,D]
  scores = (pooled . sel) / sqrt(D)              [B,P]
  attn   = softmax(scores)                       [B,P]
  action = LayerNorm(attn @ sel + pooled)        [B,D]
  sim    = (norm(pooled) . norm(protos)) / TEMP  [B,A,P]
  loss   = max(mean(-log(pos/total)), 0)   pos/total from exp(sim)

Everything derives from one dense matmul G = pooled @ protos_flat^T
([B,1024] x [1024, 2048]).  protos_flat^T is pre-scaled by 1/||proto_j||
so exp(sim) comes straight out of PSUM via one ScalarE activation with a
per-partition scale (1/(||pooled_b||*TEMP)) and a fused row-sum (total).

Sharding: data-parallel over batch, 2048 rows/core on 8 cores; prototypes
replicated.  The host SORTS the batch by app_type_id before sharding (a
pure permutation, undone on unshard), so each 128-row tile's rows select
prototypes from only 1-2 contiguous 128-column chunks ("resident window").
The attention softmax, mask+reduce ops, and the second (attention-weighted)
matmul run only on that window instead of all 2048 columns.

Engine-placement notes:
  - All ScalarE activations stay inside ONE table set
    (natural_log_exp_and_others: exp/ln/square/copy/identity).  rsqrt and
    sqrt are computed as exp(+-0.5*ln(x)) to avoid ~1.3us table reloads.
  - pooled/protos are fed as bf16 from the host so transposed operand
    layouts come straight from DMA-transpose (2-byte dtype requirement);
    no PE transposes or PSUM bounce copies for them.
  - Squares for norms use VectorE scalar_tensor_tensor self-multiplies
    with fused accumulation (bf16 4x mode).

Per-core loss partial is summed on host during unshard.
"""

import numpy as np

import concourse.bass as bass
import concourse.tile as tile
from concourse import mybir
from concourse.bass import MemorySpace
from concourse.bass_utils import run_bass_kernel_spmd
from concourse.masks import make_identity

B, A, P, D = 16384, 64, 32, 1024
NCORES = 8
BL = B // NCORES              # rows per core
AP_ = A * P                   # 2048 flattened prototypes
NDC = D // 128                # 8 contraction chunks
NJC = AP_ // 128              # 16 prototype-row chunks
TEMP = 0.07
SQD = float(np.sqrt(D))
LN_EPS = 1e-5

F32 = mybir.dt.float32
BF16 = mybir.dt.bfloat16
F8 = mybir.dt.float8e4
AF = mybir.ActivationFunctionType
OP = mybir.AluOpType
AX = mybir.AxisListType


def _bcast_row(ap, parts, n):
    """AP reading a [1, n] DRAM view broadcast across `parts` partitions."""
    return bass.AP(tensor=ap.tensor, offset=ap.offset, ap=[[0, parts], [1, n]])


def build_nc(bl=BL, jsets=None, trivial_gamma=False, trivial_beta=False,
             split_waits=True):
    """jsets: per-batch-tile (jc_lo, jc_hi) resident prototype-chunk range
    (inclusive).  None -> dense (0, NJC-1) for every tile."""
    nbt = bl // 128
    if jsets is None:
        jsets = [(0, NJC - 1)] * nbt
    assert len(jsets) == nbt
    wmax = max(hi - lo + 1 for lo, hi in jsets) * 128

    nc = bass.Bass()

    pooled = nc.declare_dram_parameter("pooled_bf", [bl, D], BF16, isOutput=False)
    pooledT = nc.declare_dram_parameter("pooledT8", [D, bl], F8, isOutput=False)
    protos = nc.declare_dram_parameter("protos_bf", [AP_, D], BF16, isOutput=False)
    protosT = nc.declare_dram_parameter("protosT8", [D, AP_], F8, isOutput=False)
    idsf = nc.declare_dram_parameter("idsf", [bl, 1], F32, isOutput=False)
    jblock = nc.declare_dram_parameter("jblock", [1, AP_], F32, isOutput=False)
    gamma = nc.declare_dram_parameter("gamma", [1, D], F32, isOutput=False)
    beta = nc.declare_dram_parameter("beta", [1, D], F32, isOutput=False)
    act_out = nc.declare_dram_parameter("act_out", [bl, D], F32, isOutput=True)
    loss_out = nc.declare_dram_parameter("loss_out", [1, 1], F32, isOutput=True)

    norms_dram = nc.dram_tensor("norms_scratch", [2, AP_], F32)

    with tile.TileContext(nc) as tc:
        with (
            tc.tile_pool(name="consts", bufs=1) as consts,
            tc.tile_pool(name="protop", bufs=1) as protop,
            tc.tile_pool(name="work", bufs=4) as work,
            tc.tile_pool(name="big", bufs=4 if wmax <= 512 else 1) as big,
            tc.tile_pool(name="small", bufs=6) as small,
            tc.tile_pool(name="psA", bufs=2, space=MemorySpace.PSUM) as psA,
            tc.tile_pool(name="psB", bufs=2, space=MemorySpace.PSUM) as psB,
            tc.tile_pool(name="psT", bufs=2, space=MemorySpace.PSUM) as psT,
        ):
            # ---------------- constants ----------------
            ident = consts.tile([128, 128], BF16, tag="ident")
            make_identity(nc, ident[:])
            ident_f = consts.tile([128, 128], F32, tag="ident_f")
            make_identity(nc, ident_f[:])
            jblock_b = consts.tile([128, AP_], BF16, tag="jblock_b")
            nc.gpsimd.dma_start(out=jblock_b[:], in_=_bcast_row(jblock[:, :], 128, AP_))
            if not trivial_gamma:
                gamma_b = consts.tile([128, D], F32, tag="gamma_b")
                nc.sync.dma_start(out=gamma_b[:], in_=_bcast_row(gamma[:, :], 128, D))
            if not trivial_beta:
                beta_b = consts.tile([128, D], F32, tag="beta_b")
                nc.sync.dma_start(out=beta_b[:], in_=_bcast_row(beta[:, :], 128, D))
            ones_col = consts.tile([128, 1], F32, tag="ones_col")
            nc.vector.memset(ones_col[:], 1.0)
            ones_row = consts.tile([1, 128], BF16, tag="ones_row")
            nc.vector.memset(ones_row[:], 1.0)
            loss_acc = consts.tile([128, 1], F32, tag="loss_acc")
            nc.vector.memset(loss_acc[:], 0.0)
            b_eps10 = consts.tile([128, 1], F32, tag="b_eps10")
            nc.vector.memset(b_eps10[:], 1e-10)
            b_lneps = consts.tile([128, 1], F32, tag="b_lneps")
            nc.vector.memset(b_lneps[:], LN_EPS)
            b_nlt = consts.tile([128, 1], F32, tag="b_nlt")
            nc.vector.memset(b_nlt[:], float(-np.log(TEMP)))

            # ---------------- prototype / pooled prep ----------------
            # proto_nat[jc]: [128 j, 1024 d] bf16 (action-matmul rhs)
            # protoT[dc]:    [128 d, 2048 j] bf16 via DMA-transpose, then
            #                columns pre-scaled by 1/||proto_j||
            # poolT_all[dc]: [128 d, bl b] bf16 via DMA-transpose
            proto_nat = [protop.tile([128, D], BF16, tag=f"pn{jc}", name=f"pn{jc}")
                         for jc in range(NJC)]
            protoT8 = protop.tile([128, NDC, AP_], F8, tag="protoT8")
            poolT8 = protop.tile([128, NDC, bl], F8, tag="poolT8")
            invn_all = consts.tile([128, NJC], F32, tag="invn_all")
            norm_all = consts.tile([128, NJC], F32, tag="norm_all")

            for dc in range(NDC):
                nc.scalar.dma_start(
                    out=protoT8[:, dc, :], in_=protosT[dc * 128:(dc + 1) * 128, :])
                nc.scalar.dma_start(
                    out=poolT8[:, dc, :], in_=pooledT[dc * 128:(dc + 1) * 128, :])

            for jc in range(NJC):
                nc.sync.dma_start(out=proto_nat[jc][:],
                                  in_=protos[jc * 128:(jc + 1) * 128, :])
                scr = work.tile([128, D], BF16, tag="scr")
                sqs = small.tile([128, 1], F32, tag="sqs")
                if jc % 2 == 0:
                    nc.vector.scalar_tensor_tensor(
                        scr[:], proto_nat[jc][:], 1.0, proto_nat[jc][:],
                        op0=OP.bypass, op1=OP.mult, accum_out=sqs[:])
                else:
                    nc.scalar.activation(scr[:], proto_nat[jc][:], AF.Square,
                                         accum_out=sqs[:])
                lnz = small.tile([128, 1], F32, tag="lnz")
                nc.scalar.activation(lnz[:], sqs[:], AF.Ln)
                nc.scalar.activation(norm_all[:, jc:jc + 1], lnz[:], AF.Exp, scale=0.5)
                nc.scalar.activation(invn_all[:, jc:jc + 1], lnz[:], AF.Exp, scale=-0.5)

            # broadcast rows of 1/||p_j|| and ||p_j|| (both bf16):
            # transpose [128,16] -> [16,128] -> DRAM row -> partition-bcast
            invnp_b = consts.tile([128, AP_], BF16, tag="invnp_b")
            nprot_b = consts.tile([128, AP_], BF16, tag="nprot_b")
            for row, srct, dstb in ((0, invn_all, invnp_b), (1, norm_all, nprot_b)):
                tpn = psT.tile([16, 128], F32, tag="tp", name=f"tpn{row}")
                nc.tensor.transpose(tpn[:], srct[:], ident_f[:])
                t16 = consts.tile([16, 128], F32, tag=f"t16_{row}", name=f"t16_{row}")
                nc.vector.tensor_copy(t16[:], tpn[:])
                nd = norms_dram[row:row + 1, :]
                dst = bass.AP(tensor=nd.tensor, offset=nd.offset, ap=[[128, 16], [1, 128]])
                nc.sync.dma_start(out=dst, in_=t16[:])
                nc.gpsimd.dma_start(out=dstb[:], in_=_bcast_row(nd, 128, AP_))

            # ---------------- upfront pooled prep (all tiles) ----------------
            pool_nats = [protop.tile([128, D], BF16, tag=f"pnat{t}", name=f"pnat{t}")
                         for t in range(nbt)]
            ids_all = consts.tile([128, nbt], F32, tag="ids_all")
            escale_all = consts.tile([128, nbt], F32, tag="escale_all")
            for t in range(nbt):
                tb = slice(t * 128, (t + 1) * 128)
                nc.sync.dma_start(out=pool_nats[t][:], in_=pooled[tb, :])
                nc.sync.dma_start(out=ids_all[:, t:t + 1], in_=idsf[tb, :])
                scr = work.tile([128, D], BF16, tag="scr")
                sqs = small.tile([128, 1], F32, tag="sqs")
                if t % 2 == 0:
                    nc.vector.scalar_tensor_tensor(
                        scr[:], pool_nats[t][:], 1.0, pool_nats[t][:],
                        op0=OP.bypass, op1=OP.mult, accum_out=sqs[:])
                else:
                    nc.scalar.activation(scr[:], pool_nats[t][:], AF.Square,
                                         accum_out=sqs[:])
                lnz = small.tile([128, 1], F32, tag="lnz")
                nc.scalar.activation(lnz[:], sqs[:], AF.Ln)
                nc.scalar.activation(escale_all[:, t:t + 1], lnz[:], AF.Exp,
                                     scale=-0.5, bias=b_nlt[:])

            # ---------------- per-batch-tile pipeline ----------------
            # First N_EARLY tiles run on UNSCALED protoT8 so PE can start
            # before the prototype-norm chain resolves; they scale gp on DVE.
            # The in-place prescale is emitted at bt == N_EARLY (Tile's WAR
            # tracking orders it after the early tiles' raw reads).
            N_EARLY = 4
            for bt in range(nbt):
                if bt == N_EARLY:
                    for dc in range(NDC):
                        nc.vector.tensor_mul(protoT8[:, dc, :], protoT8[:, dc, :],
                                             invnp_b[:])
                jlo, jhi = jsets[bt]
                nres = jhi - jlo + 1
                W = nres * 128
                jres = slice(jlo * 128, (jhi + 1) * 128)
                bsl = slice(bt * 128, (bt + 1) * 128)

                pool_nat = pool_nats[bt]
                escale = escale_all[:, bt:bt + 1]
                ids_c = ids_all[:, bt:bt + 1]

                # G matmul, fp8 DoubleRow (2 k-tiles per step): halves of 1024
                # E_c = exp(sim) straight from PSUM; per-half row-sums
                E_c = big.tile([128, AP_], BF16, tag="E_c")
                Gs = big.tile([128, AP_], BF16, tag="Gs", name="Gs") if bt < N_EARLY else None
                tsl = small.tile([128, 2], F32, tag="tsl")
                gps = []
                for hf in range(2):
                    gp = psA.tile([128, 1024], F32, tag="gp", name=f"gp{hf}")
                    gps.append(gp)
                    jsl = slice(hf * 1024, (hf + 1) * 1024)
                    for sub in range(2):
                        for q in range(4):
                            nc.tensor.matmul(
                                gp[:, sub * 512:(sub + 1) * 512],
                                poolT8[:, 2 * q:2 * q + 2, bsl],
                                protoT8[:, 2 * q:2 * q + 2,
                                        hf * 1024 + sub * 512:hf * 1024 + (sub + 1) * 512],
                                start=(q == 0),
                                stop=(q == 3),
                                perf_mode=mybir.MatmulPerfMode.DoubleRow,
                            )
                    if bt < N_EARLY:
                        nc.vector.tensor_mul(Gs[:, jsl], gp[:], invnp_b[:, jsl])
                        nc.scalar.activation(E_c[:, jsl], Gs[:, jsl], AF.Exp,
                                             scale=escale, accum_out=tsl[:, hf:hf + 1])
                    else:
                        nc.scalar.activation(E_c[:, jsl], gp[:], AF.Exp,
                                             scale=escale, accum_out=tsl[:, hf:hf + 1])
                total = small.tile([128, 1], F32, tag="total")
                nc.vector.tensor_add(total[:], tsl[:, 0:1], tsl[:, 1:2])

                # raw G on the resident window: G = gp * ||p_j||
                Graw = big.tile([128, wmax], F32, tag="Graw")
                for r in range(nres):
                    jc = jlo + r
                    hf = jc // 8
                    off = (jc % 8) * 128
                    nc.vector.tensor_mul(
                        Graw[:, r * 128:(r + 1) * 128],
                        gps[hf][:, off:off + 128],
                        nprot_b[:, jc * 128:(jc + 1) * 128],
                    )
                # attention numerators on the window
                E_a = big.tile([128, wmax], F32, tag="E_a")
                nc.scalar.activation(E_a[:, :W], Graw[:, :W], AF.Exp, scale=1.0 / SQD)

                # pos = sum over own block of E_c (mask fused)
                Ecm = big.tile([128, wmax], BF16, tag="Ecm")
                pos = small.tile([128, 1], F32, tag="pos")
                nc.vector.scalar_tensor_tensor(
                    Ecm[:, :W], jblock_b[:, jres], ids_c, E_c[:, jres],
                    op0=OP.is_equal, op1=OP.mult, accum_out=pos[:],
                )
                # attention: masked numerators + denominator
                E_am = big.tile([128, wmax], BF16, tag="E_am")
                den = small.tile([128, 1], F32, tag="den")
                nc.vector.scalar_tensor_tensor(
                    E_am[:, :W], jblock_b[:, jres], ids_c, E_a[:, :W],
                    op0=OP.is_equal, op1=OP.mult, accum_out=den[:],
                )

                # loss_acc += ln(total + 1e-10) - ln(pos)
                lt = small.tile([128, 1], F32, tag="lt")
                nc.scalar.activation(lt[:], total[:], AF.Ln, bias=b_eps10[:])
                lp = small.tile([128, 1], F32, tag="lp")
                nc.scalar.activation(lp[:], pos[:], AF.Ln)
                dlt = small.tile([128, 1], F32, tag="dlt")
                nc.vector.tensor_sub(dlt[:], lt[:], lp[:])
                nc.vector.tensor_add(loss_acc[:], loss_acc[:], dlt[:])

                # transpose E_am -> lhsT chunks
                E_amT = big.tile([128, wmax], BF16, tag="E_amT")
                for g in range((nres + 3) // 4):
                    kn = min(4, nres - g * 4)
                    tp = psT.tile([128, 4, 128], BF16, tag="tp")
                    for k in range(kn):
                        r = g * 4 + k
                        nc.tensor.transpose(tp[:, k], E_am[:, r * 128:(r + 1) * 128], ident[:])
                    nc.vector.tensor_copy(
                        E_amT[:, g * 512:g * 512 + kn * 128].rearrange("p (k f) -> p k f", k=kn),
                        tp[:, 0:kn],
                    )

                # action matmul over resident chunks only;
                # act1 = U/den + pooled with fused row-sum (LayerNorm mean)
                recip_den = small.tile([128, 1], F32, tag="recip_den")
                nc.vector.reciprocal(recip_den[:], den[:])
                act1 = work.tile([128, D], F32, tag="act1")
                s1p = small.tile([128, 2], F32, tag="s1p")
                for dsl in range(2):
                    up = psB.tile([128, 512], F32, tag="up")
                    dslc = slice(dsl * 512, (dsl + 1) * 512)
                    for r in range(nres):
                        jc = jlo + r
                        nc.tensor.matmul(
                            up[:],
                            E_amT[:, r * 128:(r + 1) * 128],
                            proto_nat[jc][:, dslc],
                            start=(r == 0),
                            stop=(r == nres - 1),
                        )
                    nc.vector.scalar_tensor_tensor(
                        act1[:, dslc], up[:], recip_den[:], pool_nat[:][:, dslc],
                        op0=OP.mult, op1=OP.add, accum_out=s1p[:, dsl:dsl + 1],
                    )

                # LayerNorm via E[x^2]-mu^2 (single pass over act1):
                #   mu = (s1p0+s1p1)/D;  var = sumsq/D - mu^2
                #   out = act1*rstd - mu*rstd   (one fused two-scalar op)
                scr2 = work.tile([128, D], BF16, tag="scr")
                vs = small.tile([128, 1], F32, tag="vs")
                nc.scalar.activation(scr2[:], act1[:], AF.Square, accum_out=vs[:])
                s1 = small.tile([128, 1], F32, tag="s1")
                nc.vector.tensor_add(s1[:], s1p[:, 0:1], s1p[:, 1:2])
                mu = small.tile([128, 1], F32, tag="mu")
                nc.vector.tensor_scalar_mul(mu[:], s1[:], 1.0 / D)
                musq = small.tile([128, 1], F32, tag="musq")
                nc.vector.tensor_mul(musq[:], mu[:], mu[:])
                var = small.tile([128, 1], F32, tag="var")
                nc.vector.scalar_tensor_tensor(
                    var[:], vs[:], 1.0 / D, musq[:],
                    op0=OP.mult, op1=OP.subtract)
                lnv = small.tile([128, 1], F32, tag="lnv")
                nc.scalar.activation(lnv[:], var[:], AF.Ln, bias=b_lneps[:])
                rstd = small.tile([128, 1], F32, tag="rstd")
                nc.scalar.activation(rstd[:], lnv[:], AF.Exp, scale=-0.5)
                murstd = small.tile([128, 1], F32, tag="murstd")
                nc.vector.tensor_mul(murstd[:], mu[:], rstd[:])
                outt = work.tile([128, D], F32, tag="outt")
                if trivial_gamma and trivial_beta:
                    nc.vector.tensor_scalar(
                        outt[:], act1[:], rstd[:], murstd[:],
                        op0=OP.mult, op1=OP.subtract)
                else:
                    xn = work.tile([128, D], F32, tag="xn")
                    nc.vector.tensor_scalar(
                        xn[:], act1[:], rstd[:], murstd[:],
                        op0=OP.mult, op1=OP.subtract)
                    if not trivial_gamma:
                        nc.vector.tensor_mul(outt[:], xn[:], gamma_b[:])
                    else:
                        nc.vector.tensor_copy(outt[:], xn[:])
                    if not trivial_beta:
                        nc.vector.tensor_add(outt[:], outt[:], beta_b[:])
                nc.scalar.dma_start(out=act_out[bsl, :], in_=outt[:])

            # ---------------- loss partial: partition-sum ----------------
            lps = psT.tile([1, 1], F32, tag="tp")
            nc.tensor.matmul(lps[:], loss_acc[:], ones_col[:], start=True, stop=True)
            lsb = small.tile([1, 1], F32, tag="lsb")
            nc.vector.tensor_copy(lsb[:], lps[:])
            nc.sync.dma_start(out=loss_out[:, :], in_=lsb[:])

    if split_waits:
        _split_multi_waits(nc)
    return nc


def _split_multi_waits(nc, max_cmds=2):
    """This walrus build allows at most ~2 sync commands (waits+updates) per
    instruction.  Tile emits up to 3+ waits on fan-in instructions; hoist the
    excess waits onto single-wait ENGINE_NOPs placed just before, on the same
    engine (same blocking semantics, engine streams run in program order)."""
    for fn in nc.m.functions:
        for blk in fn.blocks:
            new = []
            for inst in blk.instructions:
                si = getattr(inst, "sync_info", None)
                waits = list(si.on_wait) if si is not None and si.on_wait else []
                ups = list(si.on_update) if si is not None and si.on_update else []
                budget = min(1, max(0, max_cmds - len(ups)))
                if len(waits) > budget:
                    nkeep = budget
                    extra, kept = waits[:len(waits) - nkeep], waits[len(waits) - nkeep:]
                    for w in extra:
                        nop = mybir.InstEventSemaphore(
                            name=nc.get_next_instruction_name(),
                            engine=inst.engine,
                            ins=[],
                            outs=[],
                        )
                        nop.sync_info = mybir.SyncInfo(on_wait=[w], on_update=[])
                        new.append(nop)
                    inst.sync_info = mybir.SyncInfo(on_wait=kept, on_update=ups)
                new.append(inst)
            blk.instructions = new


_NC_CACHE = {}


def _get_nc(bl=BL, jsets=None, trivial_gamma=False, trivial_beta=False):
    key = (bl, tuple(jsets) if jsets is not None else None, trivial_gamma, trivial_beta)
    if key not in _NC_CACHE:
        _NC_CACHE[key] = build_nc(bl, jsets, trivial_gamma, trivial_beta)
    return _NC_CACHE[key]


def plan_shards(app_type_ids, ncores=NCORES, bl=BL):
    """Sort batch by id, then deal the 128-row sorted tiles round-robin to
    cores (core = g % ncores, slot = g // ncores).  Tile-slot s covers nearly
    the same id range on every core, so ONE SPMD graph (with the per-slot
    union of resident chunk ranges) serves all cores."""
    ids = np.asarray(app_type_ids).astype(np.int64).reshape(-1)
    base = np.argsort(ids, kind="stable")
    ids_sorted = ids[base]
    ngt = len(ids) // 128
    nslots = ngt // ncores
    order = []
    for c in range(ncores):
        for s in range(nslots):
            g = s * ncores + c
            order.append(base[128 * g:128 * (g + 1)])
    perm = np.concatenate(order)
    jsets = []
    for s in range(nslots):
        lo_id = int(ids_sorted[128 * (s * ncores)])
        hi_id = int(ids_sorted[128 * (s * ncores + ncores - 1) + 127])
        jsets.append((lo_id * P // 128, hi_id * P // 128))
    return perm, jsets


def make_in_maps(pooled_output, app_type_ids, prototypes, ln_gamma, ln_beta,
                 perm=None, ncores=NCORES, bl=BL):
    bf16 = mybir.dt.np(BF16)
    f8 = mybir.dt.np(F8)
    protoflat = np.asarray(prototypes, dtype=np.float32).reshape(AP_, D).astype(bf16)
    protoT8 = np.ascontiguousarray(protoflat.T).astype(f8)
    jblock = (np.arange(AP_, dtype=np.int64) // P).astype(np.float32).reshape(1, AP_)
    gamma = np.asarray(ln_gamma, dtype=np.float32).reshape(1, D)
    beta = np.asarray(ln_beta, dtype=np.float32).reshape(1, D)
    pooled_bf = np.asarray(pooled_output, dtype=np.float32).astype(bf16)
    idsf = np.asarray(app_type_ids).astype(np.float32).reshape(-1, 1)
    if perm is not None:
        pooled_bf = pooled_bf[perm]
        idsf = idsf[perm]
    in_maps = []
    for c in range(ncores):
        sl = slice(c * bl, (c + 1) * bl)
        in_maps.append({
            "pooled_bf": np.ascontiguousarray(pooled_bf[sl]),
            "pooledT8": np.ascontiguousarray(pooled_bf[sl].T).astype(f8),
            "protos_bf": protoflat,
            "protosT8": protoT8,
            "idsf": np.ascontiguousarray(idsf[sl]),
            "jblock": jblock,
            "gamma": gamma,
            "beta": beta,
        })
    return in_maps


def _prep(pooled_output, app_type_ids, prototypes, ln_gamma, ln_beta):
    perm, jsets = plan_shards(app_type_ids)
    tg = bool(np.all(np.asarray(ln_gamma) == 1.0))
    tb = bool(np.all(np.asarray(ln_beta) == 0.0))
    nc = _get_nc(BL, jsets, tg, tb)
    in_maps = make_in_maps(pooled_output, app_type_ids, prototypes,
                           ln_gamma, ln_beta, perm=perm)
    return nc, in_maps, perm


def kernel(pooled_output, app_type_ids, prototypes, ln_gamma, ln_beta):
    nc, in_maps, perm = _prep(pooled_output, app_type_ids, prototypes,
                              ln_gamma, ln_beta)
    res = run_bass_kernel_spmd(nc, in_maps, core_ids=list(range(NCORES)))
    action_sorted = np.concatenate([r["act_out"] for r in res.results], axis=0)
    action = np.empty_like(action_sorted)
    action[perm] = action_sorted
    loss_sum = sum(float(r["loss_out"][0, 0]) for r in res.results)
    loss = np.float32(max(loss_sum / B, 0.0))
    return action.astype(np.float32), loss
